# revision 5
# baseline (speedup 1.0000x reference)
"""Differentiable rasterizer on 8 Trainium2 NeuronCores (Bass/Tile).

Math: for each pixel, per stroke, min over bezier samples of squared distance
is computed on the TensorEngine as a quadratic form (pixels as weights,
candidate samples as streaming columns, accumulated negated in PSUM), reduced
on DVE, then alpha-compositing is evaluated in closed form
    C = 1 + sum_s alpha_s * T_s * (c_s - 1),  T_s = prod_{j>s} (1 - alpha_j)
with log-space suffix sums via a triangular matmul. Only ln/exp ACT tables
are used (one ACT function set per kernel):
    2d   = exp(0.5 * ln(-4*m))
    sp   = softplus(arg) = ln(1 + exp(arg)),  arg = 2w - 2d
    w_s  = alpha_s * T_s = exp(arg - sp + psumE),  psumE = -sum_{j>s} sp_j

Work is pruned host-side: for each 16x8 pixel tile and stroke, only samples
with d(center, s) <= dmin_center + 2r (r = tile circumradius) can be the
argmin anywhere in the tile (triangle inequality; exact), plus an absolute
cut  d(center, s) <= r + w + 46  beyond which alpha underflows to 0 in fp32.
Strokes are permuted actives-first per tile (U / widths / colors permuted to
match as per-tile weight data). Tiles are sorted by cost and dealt
round-robin to the 8 cores so the SPMD (single-program) shapes stay tight.
"""
import os
import sys
import time

import numpy as np

sys.path.insert(0, "/opt/trn_rl_repo")

import concourse.bass as bass
import concourse.mybir as mybir
from concourse.tile import TileContext
from concourse.bass_utils import run_bass_kernel_spmd

AF = mybir.ActivationFunctionType
ALU = mybir.AluOpType
F32 = mybir.dt.float32

CS = 512
NSAMP = 50
NSTR = 64
TH, TW = 8, 16  # tile height x width (pixels)
NTY, NTX = CS // TH, CS // TW  # 64 x 32 = 2048 tiles
NTILES = NTY * NTX
NCORES = 8
NSLOTS = NTILES // NCORES  # 256
NGROUPS_FULL = NSLOTS // 8  # 32 groups of 8 slots (4 pairs)
R_TILE = float(np.hypot((TW - 1) / 2.0, (TH - 1) / 2.0))
W_MARGIN = 46.0
DUMMY_PN = 1.0e9

MAX_WAITS = 1


def _split_excess_waits(nc):
    """walrus in this build rejects >1 sync-wait per instruction; move the
    excess onto NoOps inserted before the instruction on the same engine."""
    n_split = 0
    for fn in nc.m.functions:
        for bb in fn.blocks:
            insts = list(bb.instructions)
            out = []
            changed = False
            for inst in insts:
                si = inst.sync_info
                waits = list(si.on_wait) if si is not None and si.on_wait else []
                if len(waits) > MAX_WAITS:
                    changed = True
                    extra = waits[: len(waits) - MAX_WAITS]
                    keep = waits[len(extra):]
                    for i in range(0, len(extra), MAX_WAITS):
                        nop = mybir.InstNoOp(
                            name=f"{inst.name}-ws{n_split}-{i}", ins=[], outs=[]
                        )
                        nop.engine = inst.engine
                        nop.sync_info = mybir.SyncInfo(
                            on_wait=extra[i : i + MAX_WAITS], on_update=[]
                        )
                        out.append(nop)
                    si.on_wait = keep
                    n_split += 1
                out.append(inst)
            if changed:
                bb.instructions[:] = out
    return n_split


def _sample_points(strokes):
    """Mirror the reference's fp32 bezier sampling. [N, S, 2] in pixels."""
    t = np.linspace(0.0, 1.0, NSAMP, dtype=np.float32)[:, None]
    p0, p1, p2, p3 = strokes[:, 0], strokes[:, 1], strokes[:, 2], strokes[:, 3]
    pts = (
        (1 - t[None]) ** 3 * p0[:, None]
        + 3 * (1 - t[None]) ** 2 * t[None] * p1[:, None]
        + 3 * (1 - t[None]) * t[None] ** 2 * p2[:, None]
        + t[None] ** 3 * p3[:, None]
    ).astype(np.float32)
    return pts * np.float32(CS)


def _plan_and_pack(strokes, widths, colors, n_groups):
    """Host-side pruning, tile->core assignment, and input packing."""
    pts = _sample_points(strokes)  # [N,S,2]

    # tile centers
    txc = np.arange(NTX, dtype=np.float64) * TW + (TW - 1) / 2.0
    tyc = np.arange(NTY, dtype=np.float64) * TH + (TH - 1) / 2.0
    cx, cy = np.meshgrid(txc, tyc, indexing="xy")
    centers = np.stack([cx.ravel(), cy.ravel()], -1).astype(np.float32)  # [T,2]

    dc = np.sqrt(
        ((centers[:, None, None, :] - pts[None, :, :, :]) ** 2).sum(-1)
    )  # [T,N,S] float32->64 ok
    dmin_c = dc.min(-1)
    keep = (dc <= dmin_c[:, :, None] + 2 * R_TILE) & (
        dc <= R_TILE + widths[None, :, None] + W_MARGIN
    )  # [T,N,S]
    k_tn = keep.sum(-1)  # candidates per (tile, stroke)
    n_act_t = np.maximum((k_tn > 0).sum(-1), 1)  # [T]
    k_t = np.maximum(k_tn.max(-1), 1)  # [T]

    # sort tiles: cluster by shape so per-slot max-padding stays tight
    order = np.lexsort((n_act_t, -k_t * 64 - n_act_t))  # primary: cost desc
    # per-slot (uniform across cores) shapes
    slot_tiles = order.reshape(NSLOTS, NCORES)  # slot i -> 8 tiles
    n_slot = n_act_t[slot_tiles].max(-1)
    k_slot = k_t[slot_tiles].max(-1)
    # pair-uniform shapes (slots 2p, 2p+1 share dt/reduce)
    n_pair = np.maximum(n_slot[0::2], n_slot[1::2])
    k_pair = np.maximum(k_slot[0::2], k_slot[1::2])

    npairs = NSLOTS // 2
    ksegs = []
    for p in range(npairs):
        n = int(n_pair[p])
        kseg = max(1, 256 // n)
        segs = int(np.ceil(k_pair[p] / kseg))
        kseg = int(np.ceil(k_pair[p] / segs))  # rebalance
        ksegs.append((n, kseg, segs))

    # --- pack per-core candidate tensors ---
    # column layout: per pair, per segment: [slotA strokes x kseg | slotB ...]
    # stroke-major within each slot-half.
    widths2 = (2.0 * widths).astype(np.float32)
    colors_m1 = (colors - 1.0).astype(np.float32)
    U0 = -np.triu(np.ones((NSTR, NSTR), np.float32), 1).T  # U0[j,s]=-1 if j>s

    totw = sum(2 * n * kseg * segs for (n, kseg, segs) in ksegs)
    cand = np.zeros((NCORES, 4, totw), np.float32)
    cand[:, 3, :] = -DUMMY_PN  # default: dummy columns
    ucm = np.zeros((NCORES, npairs, 128, 134), np.float32)
    w2all = np.zeros((NCORES, 128, npairs), np.float32)

    perms = np.empty((NTILES, NSTR), np.int64)
    for T in range(NTILES):
        act = np.nonzero(k_tn[T] > 0)[0]
        inact = np.nonzero(k_tn[T] == 0)[0]
        perms[T] = np.concatenate([act, inact])

    col_off = 0
    pair_meta = []
    for p in range(npairs):
        n, kseg, segs = ksegs[p]
        width_pair = 2 * n * kseg  # columns per segment
        for c in range(NCORES):
            for h in range(2):
                T = slot_tiles[2 * p + h, c]
                perm = perms[T]
                ctr = centers[T]
                # ucm: U block + colors block, half h
                rows = slice(64 * h, 64 * h + 64)
                Up = U0[np.ix_(perm, perm)]
                ucm[c, p, rows, 64 * h : 64 * h + 64] = Up
                ucm[c, p, rows, 128 + 3 * h : 131 + 3 * h] = colors_m1[perm]
                w2all[c, rows, p] = widths2[perm]
                # candidates (strokes with no candidates keep dummy columns)
                for j in range(n):
                    s_idx = perm[j]
                    if k_tn[T, s_idx] == 0:
                        continue
                    cs = np.nonzero(keep[T, s_idx])[0]
                    q = pts[s_idx, cs].astype(np.float32) - ctr[None, :]
                    for ci in range(len(cs)):
                        seg, r = divmod(ci, kseg)
                        col = (
                            col_off
                            + seg * width_pair
                            + h * (n * kseg)
                            + j * kseg
                            + r
                        )
                        qx, qy = float(q[ci, 0]), float(q[ci, 1])
                        cand[c, 0, col] = 2.0 * qx
                        cand[c, 1, col] = 2.0 * qy
                        cand[c, 2, col] = -1.0
                        cand[c, 3, col] = -(qx * qx + qy * qy)
        pair_meta.append((col_off, n, kseg, segs))
        col_off += width_pair * segs
    assert col_off == totw

    # pixel quad (tile-local, same for every tile): partition p = di*TW+dj
    dj = np.tile(np.arange(TW, dtype=np.float32), TH)
    di = np.repeat(np.arange(TH, dtype=np.float32), TW)
    xl = dj - (TW - 1) / 2.0
    yl = di - (TH - 1) / 2.0
    pixq = np.stack([xl, yl, xl * xl + yl * yl, np.ones(128, np.float32)], 0)
    pixq = pixq.astype(np.float32)  # [4,128]

    ident = np.eye(128, dtype=np.float32)

    in_maps = [
        {
            "cand": cand[c],
            "ucm": ucm[c],
            "w2all": w2all[c],
            "pixq": pixq,
            "ident": ident,
        }
        for c in range(NCORES)
    ]
    plan = {
        "pair_meta": pair_meta,
        "totw": totw,
        "npairs": npairs,
        "n_groups": n_groups,
        "slot_tiles": slot_tiles,
    }
    return in_maps, plan


def _build_program(plan):
    n_groups = plan["n_groups"]
    npairs = plan["npairs"]
    totw = plan["totw"]

    nc = bass.Bass("TRN2", target_bir_lowering=False, debug=False,
                   num_devices=NCORES)
    cand = nc.dram_tensor("cand", [4, totw], F32, kind="ExternalInput").ap()
    ucm = nc.dram_tensor("ucm", [npairs, 128, 134], F32,
                         kind="ExternalInput").ap()
    w2all_d = nc.dram_tensor("w2all", [128, npairs], F32,
                             kind="ExternalInput").ap()
    pixq_d = nc.dram_tensor("pixq", [4, 128], F32, kind="ExternalInput").ap()
    ident_d = nc.dram_tensor("ident", [128, 128], F32,
                             kind="ExternalInput").ap()
    out = nc.dram_tensor("out", [6, NGROUPS_FULL, 512], F32,
                         kind="ExternalOutput").ap()

    with TileContext(nc) as tc:
        with (
            tc.tile_pool(name="const", bufs=1) as constp,
            tc.tile_pool(name="sb", bufs=2) as sb,
            tc.tile_pool(name="sbsmall", bufs=4) as sbs,
            tc.tile_pool(name="ps", bufs=2, space="PSUM") as ps,
        ):
            pixq_t = constp.tile([4, 128], F32, tag="pixq")
            ident_t = constp.tile([128, 128], F32, tag="ident")
            w2_t = constp.tile([128, npairs], F32, tag="w2")
            nc.sync.dma_start(pixq_t[:], pixq_d[:])
            nc.sync.dma_start(ident_t[:], ident_d[:])
            nc.sync.dma_start(w2_t[:], w2all_d[:])

            for g in range(n_groups):
                mT = ps.tile([128, 512], F32, tag="mT")
                pE = ps.tile([128, 512], F32, tag="pE")
                pC = ps.tile([6, 512], F32, tag="pC")
                ucm_ts = []
                for jj in range(4):
                    p = g * 4 + jj
                    col0, n, kseg, segs = plan["pair_meta"][p]
                    wseg = 2 * n * kseg
                    ucm_t = sbs.tile([128, 134], F32, tag="ucm")
                    nc.sync.dma_start(ucm_t[:], ucm[p])
                    ucm_ts.append(ucm_t)
                    mb = sbs.tile([128, 128], F32, tag="mb")
                    for seg in range(segs):
                        cd = sbs.tile([4, wseg], F32, tag="cd")
                        nc.sync.dma_start(
                            cd[:], cand[:, col0 + seg * wseg : col0 + (seg + 1) * wseg]
                        )
                        dt = ps.tile([128, wseg], F32, tag="dt")
                        # two matmuls (N<=512 each): slot halves
                        nc.tensor.matmul(
                            dt[:, : n * kseg], pixq_t[:], cd[:, : n * kseg]
                        )
                        nc.tensor.matmul(
                            dt[:, n * kseg :], pixq_t[:], cd[:, n * kseg :]
                        )
                        dt_v = dt[:].rearrange("p (a n k) -> p a n k", a=2, n=n)
                        if seg == 0:
                            mb_v = mb[:].rearrange(
                                "p (a c) -> p a c", a=2
                            )[:, :, :n]
                            nc.vector.tensor_reduce(
                                mb_v, dt_v, axis=mybir.AxisListType.X,
                                op=ALU.max,
                            )
                        else:
                            tmpr = sbs.tile([128, 128], F32, tag="tmpr")
                            tmp_v = tmpr[:].rearrange(
                                "p (a c) -> p a c", a=2
                            )[:, :, :n]
                            nc.vector.tensor_reduce(
                                tmp_v, dt_v, axis=mybir.AxisListType.X,
                                op=ALU.max,
                            )
                            mb_v = mb[:].rearrange(
                                "p (a c) -> p a c", a=2
                            )[:, :, :n]
                            nc.vector.tensor_tensor(mb_v, mb_v, tmp_v, ALU.max)
                    if n < 64:
                        nc.gpsimd.memset(mb[:, n:64], -DUMMY_PN)
                        nc.gpsimd.memset(mb[:, 64 + n : 128], -DUMMY_PN)
                    # transpose pair -> mT[:, 128jj:...]
                    nc.tensor.transpose(
                        mT[:, 128 * jj : 128 * (jj + 1)], mb[:], ident_t[:]
                    )

                lnt = sb.tile([128, 512], F32, tag="lnt")
                s2t = sb.tile([128, 512], F32, tag="s2t")
                argt = sb.tile([128, 512], F32, tag="argt")
                ept = sb.tile([128, 512], F32, tag="ept")
                spt = sb.tile([128, 512], F32, tag="spt")
                t1 = sb.tile([128, 512], F32, tag="t1")
                t2 = sb.tile([128, 512], F32, tag="t2")
                wA = sb.tile([128, 512], F32, tag="wA")
                outS = sb.tile([6, 512], F32, tag="outS")

                nc.scalar.activation(lnt[:], mT[:], AF.Ln, scale=-4.0)
                nc.scalar.activation(s2t[:], lnt[:], AF.Exp, scale=0.5)
                for jj in range(4):
                    p = g * 4 + jj
                    nc.vector.tensor_scalar(
                        argt[:, 128 * jj : 128 * (jj + 1)],
                        s2t[:, 128 * jj : 128 * (jj + 1)],
                        w2_t[:, p : p + 1],
                        -1.0,
                        ALU.subtract,
                        ALU.mult,
                    )
                nc.scalar.activation(ept[:], argt[:], AF.Exp)
                nc.scalar.activation(spt[:], ept[:], AF.Ln, bias=1.0)
                for jj in range(4):
                    nc.tensor.matmul(
                        pE[:, 128 * jj : 128 * (jj + 1)],
                        ucm_ts[jj][:, 0:128],
                        spt[:, 128 * jj : 128 * (jj + 1)],
                    )
                nc.vector.tensor_tensor(t1[:], argt[:], spt[:], ALU.subtract)
                nc.vector.tensor_tensor(t2[:], t1[:], pE[:], ALU.add)
                nc.scalar.activation(wA[:], t2[:], AF.Exp)
                for jj in range(4):
                    nc.tensor.matmul(
                        pC[:, 128 * jj : 128 * (jj + 1)],
                        ucm_ts[jj][:, 128:134],
                        wA[:, 128 * jj : 128 * (jj + 1)],
                    )
                nc.scalar.activation(outS[:], pC[:], AF.Identity, bias=1.0)
                nc.sync.dma_start(out[:, g, :], outS[:])

    _split_excess_waits(nc)
    return nc


def _scatter(plan, core_outs):
    """Assemble per-core [6, 32, 512] outputs into the [1,3,512,512] canvas."""
    canvas = np.ones((3, CS, CS), np.float32)
    slot_tiles = plan["slot_tiles"]
    for i in range(8 * plan["n_groups"]):
        g, r = divmod(i, 8)
        jj, h = divmod(r, 2)
        for c in range(NCORES):
            T = int(slot_tiles[i, c])
            tyi, txi = divmod(T, NTX)
            blk = core_outs[c][3 * h : 3 * h + 3, g, 128 * jj : 128 * (jj + 1)]
            canvas[
                :, tyi * TH : (tyi + 1) * TH, txi * TW : (txi + 1) * TW
            ] = blk.reshape(3, TH, TW)
    return canvas[None]


_CACHE = {}


def _run(inputs, n_groups, time_reps=0):
    strokes = np.asarray(inputs["strokes"], np.float32)
    widths = np.asarray(inputs["stroke_widths"], np.float32)
    colors = np.asarray(inputs["stroke_colors"], np.float32)
    assert int(inputs["canvas_size"]) == CS

    in_maps, plan = _plan_and_pack(strokes, widths, colors, n_groups)
    nc = _build_program(plan)
    res = run_bass_kernel_spmd(nc, in_maps, list(range(NCORES)))
    outs = [res.results[c]["out"] for c in range(NCORES)]
    return _scatter(plan, outs), plan, nc, in_maps


def kernel(**inputs):
    out, _, _, _ = _run(inputs, NGROUPS_FULL)
    return out


def timed_run(inputs, n_groups=NGROUPS_FULL, reps=24, warmup=3):
    """Build+compile once, then measure steady-state per-execution wall time
    with pipelined dispatch (inputs stay device-resident; donated output
    zero-buffers are created on-device inside the jitted body)."""
    import jax
    import jax.numpy as jnp
    from jax.sharding import Mesh, PartitionSpec
    from jax.experimental.shard_map import shard_map
    from concourse import bass2jax

    strokes = np.asarray(inputs["strokes"], np.float32)
    widths = np.asarray(inputs["stroke_widths"], np.float32)
    colors = np.asarray(inputs["stroke_colors"], np.float32)
    in_maps, plan = _plan_and_pack(strokes, widths, colors, n_groups)
    nc = _build_program(plan)

    bass2jax.install_neuronx_cc_hook()
    partition_name = (
        nc.partition_id_tensor.name if nc.partition_id_tensor else None
    )
    in_names, out_names, out_avals = [], [], []
    for alloc in nc.m.functions[0].allocations:
        if not isinstance(alloc, mybir.MemoryLocationSet):
            continue
        name = alloc.memorylocations[0].name
        if alloc.kind == "ExternalInput":
            if name != partition_name:
                in_names.append(name)
        elif alloc.kind == "ExternalOutput":
            out_names.append(name)
            out_avals.append(
                jax.core.ShapedArray(
                    tuple(alloc.tensor_shape), mybir.dt.np(alloc.dtype)
                )
            )
    n_params = len(in_names)
    all_names = in_names + out_names
    if partition_name is not None:
        all_names = all_names + [partition_name]

    def _body(*args):
        operands = list(args)
        if partition_name is not None:
            operands.append(bass2jax.partition_id_tensor())
        outs = bass2jax._bass_exec_p.bind(
            *operands,
            out_avals=tuple(out_avals),
            in_names=tuple(all_names),
            out_names=tuple(out_names),
            lowering_input_output_aliases=(),
            sim_require_finite=True,
            sim_require_nnan=True,
            nc=nc,
        )
        return tuple(outs)

    devices = jax.devices()[:NCORES]
    mesh = Mesh(np.asarray(devices), ("core",))
    n_outs = len(out_names)
    sharded = jax.jit(
        shard_map(
            _body,
            mesh=mesh,
            in_specs=(PartitionSpec("core"),) * (n_params + n_outs),
            out_specs=(PartitionSpec("core"),) * n_outs,
            check_rep=False,
        ),
        donate_argnums=tuple(range(n_params, n_params + n_outs)),
        keep_unused=True,
    )
    concat_in = [
        jnp.asarray(
            np.concatenate([np.asarray(in_maps[c][n]) for c in range(NCORES)], 0)
        )
        for n in in_names
    ]
    from jax.sharding import NamedSharding

    zero_shardings = tuple(
        NamedSharding(mesh, PartitionSpec("core")) for _ in out_avals
    )
    zeros_fn = jax.jit(
        lambda: tuple(
            jnp.zeros((a.shape[0] * NCORES,) + a.shape[1:], a.dtype)
            for a in out_avals
        ),
        out_shardings=zero_shardings,
    )
    # warmup (compile + load)
    for _ in range(warmup):
        outs = sharded(*concat_in, *zeros_fn())
    jax.block_until_ready(outs)
    t0 = time.perf_counter()
    handles = []
    for _ in range(reps):
        handles.append(sharded(*concat_in, *zeros_fn()))
    jax.block_until_ready(handles)
    dt_ns = (time.perf_counter() - t0) / reps * 1e9

    out_global = np.asarray(outs[0])  # [8*6, 32, 512]
    core_outs = [out_global[6 * c : 6 * c + 6] for c in range(NCORES)]
    canvas = _scatter(plan, core_outs)
    return canvas, dt_ns, plan


if __name__ == "__main__":
    n_groups = int(os.environ.get("DR_GROUPS", NGROUPS_FULL))
    import reference as ref

    inputs = ref.setup_inputs()
    t0 = time.time()
    out, plan, nc, in_maps = _run(
        {k: np.asarray(v) if not np.isscalar(v) else v for k, v in inputs.items()},
        n_groups,
    )
    print("kernel wall time:", time.time() - t0)
    expected = np.asarray(ref.reference(**inputs))
    # compare only covered tiles
    cov = np.zeros((CS, CS), bool)
    st = plan["slot_tiles"]
    for i in range(8 * n_groups):
        for c in range(NCORES):
            T = int(st[i, c])
            tyi, txi = divmod(T, NTX)
            cov[tyi * TH : (tyi + 1) * TH, txi * TW : (txi + 1) * TW] = True
    err = np.abs(out - expected)[0][:, cov]
    print(f"covered frac: {cov.mean():.3f}  max abs err: {err.max():.3e}")
